# revision 4
# baseline (speedup 1.0000x reference)
"""Associative-embedding loss (push/pull) on 8 TRN2 NeuronCores.

Strategy (pure data parallel, hardcoded):
  - B=32 images, 8 cores -> 4 images per core.
  - Per image only 510 of the 278528 tag rows are needed, so the kernel
    never streams the tags tensor: an indirect (SWDGE) DMA gathers the
    510 rows (as 128 partitions x 4 slots x 4 floats) straight from HBM.
  - Per-person sums become tiny TensorE matmuls against a static
    keypoint->person membership matrix (kp slot s = 4p+j, person=s//17).
  - Pairwise push term: diff[i,j] = q_i + q_j - 2<m_i, m_j> built by
    three accumulating matmuls into one PSUM tile (K=4 Gram + two K=1
    rank-1 updates for the q_i / q_j broadcasts).
  - Everything that depends only on `keypoints` (visibility masks,
    counts, pair masks, output scales) is precomputed on the host - it
    is a few KB per image.

Inputs: tags [32, 278528, 4] f32, keypoints [32, 30, 17, 2] int.
Output: [32, 2] f32 (push, pull) per image.
"""

import numpy as np

import concourse.bacc as bacc
import concourse.bass as bass
import concourse.mybir as mybir
import concourse.tile as tile
from concourse.bass_utils import run_bass_kernel_spmd

B, N, D = 32, 278528, 4
NPERS, NKP = 30, 17
NFLAT = NPERS * NKP          # 510 keypoints per image
KPJ = 4                      # keypoint slots per SBUF partition (128*4=512)
NCORES = 8
IMGS = B // NCORES           # 4 images per core
EPS = 1e-6
X = mybir.AxisListType


def _build_nc():
    nc = bacc.Bacc("TRN2", target_bir_lowering=False, debug=False)
    f32 = mybir.dt.float32
    tags = nc.dram_tensor("tags", [IMGS * N, D], f32, kind="ExternalInput").ap()
    idx = nc.dram_tensor("idx", [128, IMGS * KPJ], mybir.dt.int32, kind="ExternalInput").ap()
    mvis = nc.dram_tensor("mvis", [128, IMGS * KPJ * D], f32, kind="ExternalInput").ap()
    member = nc.dram_tensor("member", [128, KPJ * NPERS], f32, kind="ExternalInput").ap()
    inv4 = nc.dram_tensor("inv4", [4, IMGS * NPERS], f32, kind="ExternalInput").ap()
    misc = nc.dram_tensor("misc", [1, IMGS * 2 * NPERS], f32, kind="ExternalInput").ap()
    hmask = nc.dram_tensor("hmask", [NPERS, IMGS * NPERS], f32, kind="ExternalInput").ap()
    scales = nc.dram_tensor("scales", [1, IMGS * 2], f32, kind="ExternalInput").ap()
    out = nc.dram_tensor("out", [1, IMGS * 2], f32, kind="ExternalOutput").ap()

    with tile.TileContext(nc) as tc:
        with (
            tc.tile_pool(name="const", bufs=1) as cpool,
            tc.tile_pool(name="work", bufs=2) as wpool,
            tc.tile_pool(name="psum", bufs=1, space="PSUM") as ppool,
        ):
            idx_t = cpool.tile([128, IMGS * KPJ], mybir.dt.int32)
            nc.sync.dma_start(idx_t[:], idx)
            mvis_t = cpool.tile([128, IMGS * KPJ * D], f32)
            nc.sync.dma_start(mvis_t[:], mvis)
            member_t = cpool.tile([128, KPJ * NPERS], f32)
            nc.sync.dma_start(member_t[:], member)
            inv4_t = cpool.tile([4, IMGS * NPERS], f32)
            nc.sync.dma_start(inv4_t[:], inv4)
            misc_t = cpool.tile([1, IMGS * 2 * NPERS], f32)
            nc.sync.dma_start(misc_t[:], misc)
            hmask_t = cpool.tile([NPERS, IMGS * NPERS], f32)
            nc.sync.dma_start(hmask_t[:], hmask)
            scales_t = cpool.tile([1, IMGS * 2], f32)
            nc.sync.dma_start(scales_t[:], scales)
            ones4_t = cpool.tile([4, 1], f32)
            nc.vector.memset(ones4_t[:], 1.0)
            ones30_t = cpool.tile([NPERS, 1], f32)
            nc.vector.memset(ones30_t[:], 1.0)
            ones1_t = cpool.tile([1, NPERS], f32)
            nc.vector.memset(ones1_t[:], 1.0)
            res_t = cpool.tile([1, IMGS * 2], f32)

            for b in range(IMGS):
                c30 = slice(b * NPERS, (b + 1) * NPERS)
                cnt_sl = slice(b * 2 * NPERS, b * 2 * NPERS + NPERS)
                invcv_sl = slice(b * 2 * NPERS + NPERS, (b + 1) * 2 * NPERS)
                # 1) gather the 512 tag rows for this image; HW indirect DMA
                # consumes ONE index per partition, so issue one gather per
                # keypoint slot column j.
                g_t = wpool.tile([128, KPJ * D], f32, tag="g")
                for j in range(KPJ):
                    nc.gpsimd.indirect_dma_start(
                        out=g_t[:, j * D:(j + 1) * D],
                        out_offset=None,
                        in_=tags,
                        in_offset=bass.IndirectOffsetOnAxis(
                            ap=idx_t[:, b * KPJ + j:b * KPJ + j + 1], axis=0
                        ),
                    )
                # 2) masked tags and their squared norms
                tm_t = wpool.tile([128, KPJ * D], f32, tag="tm")
                nc.vector.tensor_mul(
                    tm_t[:], g_t[:], mvis_t[:, b * KPJ * D:(b + 1) * KPJ * D]
                )
                sq_t = wpool.tile([128, KPJ * D], f32, tag="sq")
                nc.vector.tensor_mul(sq_t[:], tm_t[:], tm_t[:])
                s2_t = wpool.tile([128, KPJ], f32, tag="s2")
                nc.vector.reduce_sum(
                    s2_t[:], sq_t[:].rearrange("p (j d) -> p j d", d=D), axis=X.X
                )
                # 3) per-person sums via membership matmuls
                msum_p = ppool.tile([4, NPERS], f32, space="PSUM", tag="msum")
                s2sum_p = ppool.tile([1, NPERS], f32, space="PSUM", tag="s2sum")
                for j in range(KPJ):
                    mem_j = member_t[:, j * NPERS:(j + 1) * NPERS]
                    nc.tensor.matmul(
                        out=msum_p[:],
                        lhsT=tm_t[:, j * D:(j + 1) * D],
                        rhs=mem_j,
                        start=(j == 0),
                        stop=(j == KPJ - 1),
                    )
                    nc.tensor.matmul(
                        out=s2sum_p[:],
                        lhsT=s2_t[:, j:j + 1],
                        rhs=mem_j,
                        start=(j == 0),
                        stop=(j == KPJ - 1),
                    )
                # 4) means, q = |mean|^2
                meanT_t = wpool.tile([4, NPERS], f32, tag="meanT")
                nc.vector.tensor_mul(meanT_t[:], msum_p[:], inv4_t[:, c30])
                sqmT_t = wpool.tile([4, NPERS], f32, tag="sqmT")
                nc.vector.tensor_mul(sqmT_t[:], meanT_t[:], meanT_t[:])
                q_p = ppool.tile([1, NPERS], f32, space="PSUM", tag="q")
                nc.tensor.matmul(
                    out=q_p[:], lhsT=ones4_t[:], rhs=sqmT_t[:], start=True, stop=True
                )
                # 5) pull row = (S2 - cnt*q) * valid/safe_cnt ; then sum
                t1_t = wpool.tile([1, NPERS], f32, tag="t1")
                nc.vector.tensor_mul(t1_t[:], q_p[:], misc_t[0:1, cnt_sl])
                t2_t = wpool.tile([1, NPERS], f32, tag="t2")
                nc.vector.tensor_sub(t2_t[:], s2sum_p[:], t1_t[:])
                pullrow_t = wpool.tile([1, NPERS], f32, tag="pullrow")
                nc.vector.tensor_mul(pullrow_t[:], t2_t[:], misc_t[0:1, invcv_sl])
                nc.vector.reduce_sum(
                    res_t[0:1, 2 * b + 1:2 * b + 2], pullrow_t[:], axis=X.X
                )
                # 6) diff[i,j] = q_i + q_j - 2<m_i, m_j>, three accumulating matmuls
                n2meanT_t = wpool.tile([4, NPERS], f32, tag="n2meanT")
                nc.vector.tensor_scalar_mul(n2meanT_t[:], meanT_t[:], -2.0)
                qrow_t = wpool.tile([1, NPERS], f32, tag="qrow")
                nc.vector.tensor_copy(qrow_t[:], q_p[:])
                diff_p = ppool.tile([NPERS, NPERS], f32, space="PSUM", tag="diff")
                nc.tensor.matmul(
                    out=diff_p[:], lhsT=n2meanT_t[:], rhs=meanT_t[:],
                    start=True, stop=False,
                )
                nc.tensor.matmul(
                    out=diff_p[:], lhsT=qrow_t[:], rhs=ones1_t[:],
                    start=False, stop=False,
                )
                nc.tensor.matmul(
                    out=diff_p[:], lhsT=ones1_t[:], rhs=qrow_t[:],
                    start=False, stop=True,
                )
                # 7) push = sum exp(-diff) * (diff != 0) * hostmask
                e_t = wpool.tile([NPERS, NPERS], f32, tag="e")
                nc.scalar.activation(
                    e_t[:], diff_p[:], mybir.ActivationFunctionType.Exp,
                    bias=0.0, scale=-1.0,
                )
                mask2_t = wpool.tile([NPERS, NPERS], f32, tag="mask2")
                nc.vector.scalar_tensor_tensor(
                    mask2_t[:], diff_p[:], 0.0, hmask_t[:, c30],
                    op0=mybir.AluOpType.not_equal, op1=mybir.AluOpType.mult,
                )
                contrib_t = wpool.tile([NPERS, NPERS], f32, tag="contrib")
                nc.vector.tensor_mul(contrib_t[:], e_t[:], mask2_t[:])
                pushrow_t = wpool.tile([NPERS, 1], f32, tag="pushrow")
                nc.vector.reduce_sum(pushrow_t[:], contrib_t[:], axis=X.X)
                ptot_p = ppool.tile([1, 1], f32, space="PSUM", tag="ptot")
                nc.tensor.matmul(
                    out=ptot_p[:], lhsT=pushrow_t[:], rhs=ones30_t[:],
                    start=True, stop=True,
                )
                nc.vector.tensor_copy(res_t[0:1, 2 * b:2 * b + 1], ptot_p[:])

            nc.vector.tensor_mul(res_t[:], res_t[:], scales_t[:])
            nc.sync.dma_start(out, res_t[:])

    nc.compile()
    return nc


_NC_CACHE = None


def _get_nc():
    global _NC_CACHE
    if _NC_CACHE is None:
        _NC_CACHE = _build_nc()
    return _NC_CACHE


def _static_member() -> np.ndarray:
    member = np.zeros((128, KPJ * NPERS), dtype=np.float32)
    for p in range(128):
        for j in range(KPJ):
            s = KPJ * p + j
            if s < NFLAT:
                member[p, j * NPERS + (s // NKP)] = 1.0
    return member


def _host_prep(tags: np.ndarray, keypoints: np.ndarray):
    """Build the per-core input maps. tags [B,N,D] f32, keypoints [B,30,17,2]."""
    kp_idx = keypoints[..., 0].reshape(B, NFLAT).astype(np.int64)
    kp_vis = (keypoints[..., 1] > 0).reshape(B, NFLAT)
    member = _static_member()

    in_maps = []
    for c in range(NCORES):
        tags_flat = np.ascontiguousarray(
            tags[c * IMGS:(c + 1) * IMGS].reshape(IMGS * N, D), dtype=np.float32
        )
        idx = np.zeros((128, IMGS * KPJ), dtype=np.int32)
        mvis = np.zeros((128, IMGS * KPJ * D), dtype=np.float32)
        inv4 = np.zeros((4, IMGS * NPERS), dtype=np.float32)
        misc = np.zeros((1, IMGS * 2 * NPERS), dtype=np.float32)
        hmask = np.zeros((NPERS, IMGS * NPERS), dtype=np.float32)
        scales = np.zeros((1, IMGS * 2), dtype=np.float32)
        for lb in range(IMGS):
            gb = c * IMGS + lb
            fidx = kp_idx[gb]            # [510]
            fvis = kp_vis[gb]            # [510]
            slot_idx = np.zeros(128 * KPJ, dtype=np.int64)
            slot_vis = np.zeros(128 * KPJ, dtype=np.float32)
            slot_idx[:NFLAT] = fidx + lb * N
            slot_vis[:NFLAT] = fvis.astype(np.float32)
            idx[:, lb * KPJ:(lb + 1) * KPJ] = slot_idx.reshape(128, KPJ)
            mvis[:, lb * KPJ * D:(lb + 1) * KPJ * D] = np.repeat(
                slot_vis.reshape(128, KPJ), D, axis=1
            )
            vis_pk = fvis.reshape(NPERS, NKP)
            cnt = vis_pk.sum(axis=1).astype(np.float32)
            valid = cnt > 0
            safe_cnt = np.maximum(cnt, 1.0)
            inv4[:, lb * NPERS:(lb + 1) * NPERS] = (1.0 / safe_cnt)[None, :]
            misc[0, lb * 2 * NPERS:lb * 2 * NPERS + NPERS] = cnt
            misc[0, lb * 2 * NPERS + NPERS:(lb + 1) * 2 * NPERS] = valid / safe_cnt
            upper = np.triu(np.ones((NPERS, NPERS), dtype=bool), 1)
            hmask[:, lb * NPERS:(lb + 1) * NPERS] = (
                upper & valid[:, None] & valid[None, :]
            ).astype(np.float32)
            n = valid.sum().astype(np.float32)
            scales[0, 2 * lb] = 1.0 / ((n - 1.0) * n + EPS)
            scales[0, 2 * lb + 1] = 1.0 / (n + EPS)
        in_maps.append(
            {
                "tags": tags_flat,
                "idx": idx,
                "mvis": mvis,
                "member": member,
                "inv4": inv4,
                "misc": misc,
                "hmask": hmask,
                "scales": scales,
            }
        )
    return in_maps


def kernel(tags: np.ndarray, keypoints: np.ndarray) -> np.ndarray:
    tags = np.asarray(tags, dtype=np.float32)
    keypoints = np.asarray(keypoints)
    nc = _get_nc()
    in_maps = _host_prep(tags, keypoints)
    res = run_bass_kernel_spmd(nc, in_maps, core_ids=list(range(NCORES)))
    outs = [np.asarray(r["out"]).reshape(IMGS, 2) for r in res.results]
    return np.concatenate(outs, axis=0)
